# revision 10
# baseline (speedup 1.0000x reference)
"""LSEP loss kernel for Trainium2 (8 NeuronCores, SPMD data-parallel).

loss = log1p( sum_i [ (sum_{c: t=0} exp(x_ic)) * (sum_{c: t=1} exp(-x_ic)) ] )

Strategy (v5, hybrid a/b forms, software-pipelined):  shard batch across
8 cores (4096 rows each); partition p holds samples [32p, 32p+32)
contiguous.  Chunks alternate between two computation forms so that the
ScalarE (ACT) and DVE engine loads balance at ~60us each, both under the
~80us DMA streaming floor:

a-form (ScalarE-heavy; masked-exp trick), 13 samples:
  a = x - 50*t                       (DVE STT, f32, chunk-wide)
  s_neg_k = sum exp(a)               (ACT per sample, accum_out)
  s_pos_k = sum exp(-a - 50)         (ACT per sample, scale=-1 bias=-50)

b-form (DVE-heavy; exact sign-flip masking), 19 samples:
  b = x ^ (t << 31)  = (-1)^t * x    (DVE STT, i32 bit domain, chunk-wide)
  u_k  = sum exp(b)  (= s_neg+s_pos) (ACT per sample, accum_out; e kept bf16)
  sp_k = sum (t*1.0) * e             (DVE STT per sample, accum_out)
  s_neg_k = u_k - sp_k               (epilogue)

The per-sample sp accumulations are EMITTED two chunks late: DVE executes
in program order, so placing sp right after its producing ACT would stall
DVE (head-of-line) while future chunks' combines are already runnable.

Epilogue: prod = s_neg*s_pos per sample, reduce, DMA [128,1] partial per
core; host sums the 1024x8 partials and applies log1p.

HW facts this design is built on (measured via ntff traces):
 - DVE: 1 elem/lane/cycle @0.96GHz for ALL dtypes (no 16-bit speedup).
 - ACT: ~1128ns per [128,1000] exp + 278ns accumulator read + ~220ns issue.
 - GpSimd: cannot run TensorScalarPtr (ISA); TT/CAST ~0.4 eff (too slow).
 - DMA: ~420 GB/s steady on the sync HWDGE ring; 32.77MB/core => ~78us floor.
"""

import numpy as np

BATCH = 32768
C = 1000
N_CORES = 8
ROWS = BATCH // N_CORES          # 4096 rows per core
P = 128                          # SBUF partitions
SPR = ROWS // P                  # 32 samples per partition
# small chunks at both ends: fast pipeline ramp-in AND a short tail
CHUNKS = [1, 1, 1] + [2] * 13 + [1, 1, 1]  # sum == 32
NA, NB = 13, 19                  # a-form/b-form sample split (ACT vs DVE)
SP_LAG = 2                       # emit sp-accums this many chunks late

_CACHE = {}


def _build_nc():
    import concourse.bacc as bacc
    import concourse.mybir as mybir
    from concourse.tile import TileContext

    f32 = mybir.dt.float32
    bf16 = mybir.dt.bfloat16
    i32 = mybir.dt.int32
    Exp = mybir.ActivationFunctionType.Exp
    Alu = mybir.AluOpType

    assert sum(CHUNKS) == SPR
    wmax = max(CHUNKS) * C

    nc = bacc.Bacc()
    x = nc.declare_dram_parameter("input", [ROWS, C], i32, isOutput=False)
    t = nc.declare_dram_parameter("target", [ROWS, C], i32, isOutput=False)
    out = nc.declare_dram_parameter("partial", [P, 1], f32, isOutput=True)

    xv = x.rearrange("(p s) c -> p (s c)", p=P)
    tv = t.rearrange("(p s) c -> p (s c)", p=P)

    def stt_shift_xor(out_ap, t_ap, x_ap):
        # b = (t << 31) ^ x.  walrus birverifier requires bitvec-op
        # immediates to be integer-typed and match src/dst dtype.
        eng = nc.vector
        eng.add_instruction(
            mybir.InstTensorScalarPtr(
                name=nc.get_next_instruction_name(),
                is_scalar_tensor_tensor=True,
                op0=Alu.logical_shift_left,
                op1=Alu.bitwise_xor,
                ins=[
                    eng.lower_ap(t_ap),
                    mybir.ImmediateValue(dtype=i32, value=31),
                    eng.lower_ap(x_ap),
                ],
                outs=[eng.lower_ap(out_ap)],
            )
        )

    # alternate chunk forms, keeping sample counts near NA:NB
    forms = []
    na = nb = 0
    for ncols in CHUNKS:
        if na * NB <= nb * NA:
            forms.append("a")
            na += ncols
        else:
            forms.append("b")
            nb += ncols

    with TileContext(nc) as tc:
        with (
            tc.tile_pool(name="io", bufs=6) as io,
            tc.tile_pool(name="acc", bufs=1) as accp,
        ):
            sn = accp.tile([P, SPR], f32)     # s_neg (a-form) / u (b-form)
            sta = accp.tile([P, SPR], f32)    # s_pos, a-form cols (ACT writes)
            stb = accp.tile([P, SPR], f32)    # s_pos, b-form cols (DVE writes)
            scr_a = accp.tile([P, C], bf16)   # discarded ACT#2 main out
            scr_s = accp.tile([P, C], bf16)   # discarded sp-accum main out
            bneg = accp.tile([P, 1], f32)     # bias AP holding -50.0
            nc.vector.memset(bneg[:], -50.0)

            def emit_sp(pend):
                for k, tt_s, esl in pend:
                    nc.vector.scalar_tensor_tensor(
                        scr_s[:], tt_s, 1.0, esl,
                        op0=Alu.mult, op1=Alu.mult,
                        accum_out=stb[:, k : k + 1],
                    )

            pending = []  # deferred sp-accums: list of lists per chunk
            off = 0
            for ci, ncols in enumerate(CHUNKS):
                w = ncols * C
                form = forms[ci]
                xt = io.tile([P, wmax], i32, tag="x")
                tt = io.tile([P, wmax], i32, tag="t")
                bt = io.tile([P, wmax], i32, tag="b")
                nc.sync.dma_start(tt[:, :w], tv[:, off * C : off * C + w])
                nc.sync.dma_start(xt[:, :w], xv[:, off * C : off * C + w])
                if form == "a":
                    # a = t*(-50) + x, chunk-wide in f32
                    nc.vector.scalar_tensor_tensor(
                        bt[:, :w].bitcast(f32), tt[:, :w], -50.0,
                        xt[:, :w].bitcast(f32), op0=Alu.mult, op1=Alu.add,
                    )
                    for j in range(ncols):
                        k = off + j
                        asl = bt[:, j * C : (j + 1) * C].bitcast(f32)
                        nc.scalar.activation(
                            scr_a[:], asl, Exp, accum_out=sn[:, k : k + 1]
                        )
                        nc.scalar.activation(
                            scr_a[:], asl, Exp, scale=-1.0, bias=bneg[:],
                            accum_out=sta[:, k : k + 1],
                        )
                else:
                    et = io.tile([P, wmax], bf16, tag="e")
                    stt_shift_xor(bt[:, :w], tt[:, :w], xt[:, :w])
                    chunk_pend = []
                    for j in range(ncols):
                        k = off + j
                        bsl = bt[:, j * C : (j + 1) * C].bitcast(f32)
                        esl = et[:, j * C : (j + 1) * C]
                        nc.scalar.activation(
                            esl, bsl, Exp, accum_out=sn[:, k : k + 1]
                        )
                        chunk_pend.append((k, tt[:, j * C : (j + 1) * C], esl))
                    pending.append(chunk_pend)
                # flush sp-accums that are SP_LAG chunks old (ACT surely done)
                if len(pending) > SP_LAG:
                    emit_sp(pending.pop(0))
                off += ncols
            for chunk_pend in pending:
                emit_sp(chunk_pend)

            # epilogue: b-form cols: s_neg = u - sp (in sn), s_pos from stb;
            # merge stb into sta so prod = sn*sta uniformly, reduce, DMA out.
            pr = accp.tile([P, SPR], f32)
            tot = accp.tile([P, 1], f32)
            runs = []
            off = 0
            for ci, ncols in enumerate(CHUNKS):
                if forms[ci] == "b":
                    if runs and runs[-1][1] == off:
                        runs[-1][1] = off + ncols
                    else:
                        runs.append([off, off + ncols])
                off += ncols
            for k0, k1 in runs:
                nc.vector.tensor_tensor(
                    pr[:, k0:k1], sn[:, k0:k1], stb[:, k0:k1], Alu.subtract
                )
                nc.vector.tensor_copy(sn[:, k0:k1], pr[:, k0:k1])
                nc.vector.tensor_copy(sta[:, k0:k1], stb[:, k0:k1])
            nc.vector.tensor_tensor(pr[:], sn[:], sta[:], Alu.mult)
            nc.vector.reduce_sum(tot[:], pr[:], axis=mybir.AxisListType.X)
            # out-DMA on the ACT HWDGE ring: the sync ring's FIFO still
            # holds input-DMA completions at this point
            nc.scalar.dma_start(out[:], tot[:])
    nc.compile()
    return nc


def _get_nc():
    if "nc" not in _CACHE:
        _CACHE["nc"] = _build_nc()
    return _CACHE["nc"]


def kernel(input, target):
    from concourse.bass_utils import run_bass_kernel_spmd

    x = np.ascontiguousarray(np.asarray(input, dtype=np.float32))
    t = np.ascontiguousarray(np.asarray(target, dtype=np.int32))
    assert x.shape == (BATCH, C) and t.shape == (BATCH, C)
    xi = x.view(np.int32)   # raw-bits view; b-form flips the sign bit via xor

    nc = _get_nc()
    in_maps = [
        {
            "input": xi[i * ROWS : (i + 1) * ROWS],
            "target": t[i * ROWS : (i + 1) * ROWS],
        }
        for i in range(N_CORES)
    ]
    res = run_bass_kernel_spmd(nc, in_maps, list(range(N_CORES)))
    total = 0.0
    for r in res.results:
        total += float(np.sum(r["partial"].astype(np.float64)))
    return np.asarray([np.log1p(total)], dtype=np.float32)


# revision 11
# speedup vs baseline: 1.0949x; 1.0949x over previous
"""LSEP loss kernel for Trainium2 (8 NeuronCores, SPMD data-parallel).

loss = log1p( sum_i [ (sum_{c: t=0} exp(x_ic)) * (sum_{c: t=1} exp(-x_ic)) ] )

Strategy (v6, per-chunk mixed a/b forms):  shard batch across 8 cores
(4096 rows each); partition p holds samples [32p, 32p+32) contiguous.
Every 2-sample chunk processes ONE sample in each form, so the ScalarE
(ACT) and DVE loads are balanced inside every chunk period (no phase
alternation -> no cross-engine convoys):

a-form (ScalarE: 2 exps; masked-exp trick):
  a_k = x - 50*t                     (DVE STT, f32, per sample)
  s_neg_k = sum exp(a)               (ACT, accum_out)
  s_pos_k = sum exp(-a - 50)         (ACT, scale=-1 bias=-50)

b-form (DVE: masked accumulate; exact sign-flip):
  b_k = x ^ (t << 31) = (-1)^t * x   (DVE STT, i32 bit domain, per sample)
  u_k  = sum exp(b)                  (ACT, accum_out; e kept bf16)
  sp_k = sum (t*1.0) * e             (DVE STT, accum_out, emitted 1 chunk late)
  s_neg_k = u_k - sp_k               (epilogue)

Per 2-sample chunk (DMA period ~4.7us @ ~420GB/s): DVE ~3.7us, ACT ~4.2us.
Ramp singles lean b-form (short first chain), tail singles a-form (shortest
drain chain).  Epilogue: prod = s_neg*s_pos, reduce, DMA [128,1] partial;
host sums 1024x8 partials and applies log1p.

HW facts (measured via ntff traces): DVE 1 elem/lane/cycle @0.96GHz for all
dtypes; ACT ~1128ns/[128,1000] exp + 278ns accumulator read; GpSimd can't
run TensorScalarPtr; DMA ~420 GB/s steady => 32.77MB/core ~ 78us floor.
"""

import numpy as np

BATCH = 32768
C = 1000
N_CORES = 8
ROWS = BATCH // N_CORES          # 4096 rows per core
P = 128                          # SBUF partitions
SPR = ROWS // P                  # 32 samples per partition
# small chunks at both ends: fast ramp-in AND a short drain chain
CHUNKS = [1, 1, 1] + [2] * 13 + [1, 1, 1]  # sum == 32
# per-sample forms: ramp singles b-form, middle chunks mixed (a,b), tail
# singles a-form.
def _default_forms():
    forms = []
    for ci, ncols in enumerate(CHUNKS):
        if ncols == 1:
            forms.append("b" if ci < 3 else "a")
        else:
            forms.append("ab")
    return forms

_CACHE = {}


def _build_nc():
    import concourse.bacc as bacc
    import concourse.mybir as mybir
    from concourse.tile import TileContext

    f32 = mybir.dt.float32
    bf16 = mybir.dt.bfloat16
    i32 = mybir.dt.int32
    Exp = mybir.ActivationFunctionType.Exp
    Alu = mybir.AluOpType

    assert sum(CHUNKS) == SPR
    wmax = max(CHUNKS) * C
    forms = _default_forms()

    nc = bacc.Bacc()
    x = nc.declare_dram_parameter("input", [ROWS, C], i32, isOutput=False)
    t = nc.declare_dram_parameter("target", [ROWS, C], i32, isOutput=False)
    out = nc.declare_dram_parameter("partial", [P, 1], f32, isOutput=True)

    xv = x.rearrange("(p s) c -> p (s c)", p=P)
    tv = t.rearrange("(p s) c -> p (s c)", p=P)

    def stt_shift_xor(out_ap, t_ap, x_ap):
        # b = (t << 31) ^ x.  walrus birverifier requires bitvec-op
        # immediates to be integer-typed and match src/dst dtype.
        eng = nc.vector
        eng.add_instruction(
            mybir.InstTensorScalarPtr(
                name=nc.get_next_instruction_name(),
                is_scalar_tensor_tensor=True,
                op0=Alu.logical_shift_left,
                op1=Alu.bitwise_xor,
                ins=[
                    eng.lower_ap(t_ap),
                    mybir.ImmediateValue(dtype=i32, value=31),
                    eng.lower_ap(x_ap),
                ],
                outs=[eng.lower_ap(out_ap)],
            )
        )

    bcols = []
    with TileContext(nc) as tc:
        with (
            tc.tile_pool(name="io", bufs=6) as io,
            tc.tile_pool(name="acc", bufs=1) as accp,
        ):
            sn = accp.tile([P, SPR], f32)     # s_neg (a-form) / u (b-form)
            sta = accp.tile([P, SPR], f32)    # s_pos, a-form cols (ACT writes)
            stb = accp.tile([P, SPR], f32)    # s_pos, b-form cols (DVE writes)
            scr_a = accp.tile([P, C], bf16)   # discarded ACT#2 main out
            scr_s = accp.tile([P, C], bf16)   # discarded sp-accum main out
            bneg = accp.tile([P, 1], f32)     # bias AP holding -50.0
            nc.vector.memset(bneg[:], -50.0)

            def emit_sp(pend):
                for k, tt_s, esl in pend:
                    nc.vector.scalar_tensor_tensor(
                        scr_s[:], tt_s, 1.0, esl,
                        op0=Alu.mult, op1=Alu.mult,
                        accum_out=stb[:, k : k + 1],
                    )

            pending = []  # deferred per-chunk sp-accum args
            off = 0
            for ci, ncols in enumerate(CHUNKS):
                w = ncols * C
                xt = io.tile([P, wmax], i32, tag="x")
                tt = io.tile([P, wmax], i32, tag="t")
                bt = io.tile([P, wmax], i32, tag="b")
                et = io.tile([P, wmax], bf16, tag="e")
                nc.sync.dma_start(tt[:, :w], tv[:, off * C : off * C + w])
                nc.sync.dma_start(xt[:, :w], xv[:, off * C : off * C + w])
                chunk_pend = []
                for j, f in enumerate(forms[ci]):
                    k = off + j
                    lo, hi = j * C, (j + 1) * C
                    if f == "a":
                        nc.vector.scalar_tensor_tensor(
                            bt[:, lo:hi].bitcast(f32), tt[:, lo:hi], -50.0,
                            xt[:, lo:hi].bitcast(f32),
                            op0=Alu.mult, op1=Alu.add,
                        )
                        asl = bt[:, lo:hi].bitcast(f32)
                        nc.scalar.activation(
                            scr_a[:], asl, Exp, accum_out=sn[:, k : k + 1]
                        )
                        nc.scalar.activation(
                            scr_a[:], asl, Exp, scale=-1.0, bias=bneg[:],
                            accum_out=sta[:, k : k + 1],
                        )
                    else:
                        bcols.append(k)
                        stt_shift_xor(bt[:, lo:hi], tt[:, lo:hi], xt[:, lo:hi])
                        esl = et[:, lo:hi]
                        nc.scalar.activation(
                            esl, bt[:, lo:hi].bitcast(f32), Exp,
                            accum_out=sn[:, k : k + 1],
                        )
                        chunk_pend.append((k, tt[:, lo:hi], esl))
                if chunk_pend:
                    pending.append(chunk_pend)
                # flush sp-accums one chunk late (their ACT is surely done)
                if len(pending) > 1:
                    emit_sp(pending.pop(0))
                off += ncols
            for chunk_pend in pending:
                emit_sp(chunk_pend)

            # epilogue: b-form cols: s_neg = u - sp (into sn), s_pos -> sta;
            # then prod = sn*sta uniformly, reduce, DMA out.
            pr = accp.tile([P, SPR], f32)
            tot = accp.tile([P, 1], f32)
            runs = []
            for k in sorted(bcols):
                if runs and runs[-1][1] == k:
                    runs[-1][1] = k + 1
                else:
                    runs.append([k, k + 1])
            for k0, k1 in runs:
                nc.vector.tensor_tensor(
                    pr[:, k0:k1], sn[:, k0:k1], stb[:, k0:k1], Alu.subtract
                )
                nc.vector.tensor_copy(sn[:, k0:k1], pr[:, k0:k1])
                nc.vector.tensor_copy(sta[:, k0:k1], stb[:, k0:k1])
            nc.vector.tensor_tensor(pr[:], sn[:], sta[:], Alu.mult)
            nc.vector.reduce_sum(tot[:], pr[:], axis=mybir.AxisListType.X)
            # out-DMA on the ACT HWDGE ring: the sync ring's FIFO still
            # holds input-DMA completions at this point
            nc.scalar.dma_start(out[:], tot[:])
    nc.compile()
    return nc


def _get_nc():
    if "nc" not in _CACHE:
        _CACHE["nc"] = _build_nc()
    return _CACHE["nc"]


def kernel(input, target):
    from concourse.bass_utils import run_bass_kernel_spmd

    x = np.ascontiguousarray(np.asarray(input, dtype=np.float32))
    t = np.ascontiguousarray(np.asarray(target, dtype=np.int32))
    assert x.shape == (BATCH, C) and t.shape == (BATCH, C)
    xi = x.view(np.int32)   # raw-bits view; b-form flips the sign bit via xor

    nc = _get_nc()
    in_maps = [
        {
            "input": xi[i * ROWS : (i + 1) * ROWS],
            "target": t[i * ROWS : (i + 1) * ROWS],
        }
        for i in range(N_CORES)
    ]
    res = run_bass_kernel_spmd(nc, in_maps, list(range(N_CORES)))
    total = 0.0
    for r in res.results:
        total += float(np.sum(r["partial"].astype(np.float64)))
    return np.asarray([np.log1p(total)], dtype=np.float32)
